# revision 20
# baseline (speedup 1.0000x reference)
"""AttentionMIL pooling kernel for 8 Trainium2 NeuronCores.

Math (per slide b): h = tanh(X @ W1^T); s = h @ w2; a = softmax(s);
out = a^T @ X, with X [N=8192, D=1024], W1 [H=256, D], w2 [H].

Strategy (single-copy, ~33.6 MB/core HBM vs 67 MB for the two-layout
baseline; measured 221.8us baseline -> ~171us -> this version):
  - Data-parallel over the slide dim: 16 slides / 8 cores = 2 per core.
  - Host ships ONLY the transposed bf16 layout xt (d on partitions, rows on
    free), swizzled as 512-row blocks. The kernel assembles blocks into
    weighted-sum groups of GRADED size [2,4,4,4,2] blocks: small groups at
    the pipeline ramp and drain (the weighted sum trails the score pipe by
    one group, so group length = pipeline skew), big groups in the middle
    (amortize fixed per-op costs on DVE/ACT). Each block is one 1 MiB DMA
    so the PE starts on the first block.
  - Scores per 512-row block: ht = W1t-stationary @ xt on PE (16 matmuls of
    F=512 into a double-buffered 2-bank fp32 PSUM tile), one tanh (ACT).
    The w2 contraction is a PE matmul whose stationary is w2 REPLICATED
    across 128 columns: out [128, 512] has every partition equal to the
    score row, i.e. scores arrive already broadcast across partitions for
    the same cost (matmul cost depends only on F). exp on ACT fuses the
    PSUM->SBUF copy and emits accum_out = sum_j exp(s_j) (softmax
    denominator; no max-subtraction needed: |s| <= ||w2||_1 ~ 13 cannot
    overflow fp32). The score tail for block i is emitted after the ht
    matmuls of block i+1 so the PE never stalls waiting on ACT.
  - Weighted sum WITHOUT a second X layout: per d-chunk k and group,
    r[p, k] += sum_j xt[p, k-chunk, j] * e[j] over the group's blocks via
    strided 3D access patterns. Chunks 0-4 via fused scalar_tensor_tensor
    on DVE (mul + free-dim reduce + [P,1] accum in one 1x op). Chunks 5-7:
    2x tensor_tensor premultiplies on DVE, then three activation-accumulate
    reduces on ACT, deferred one block so tanh/exp keep priority in the
    scalar engine's in-order queue. Measured steady state: DVE ~145us,
    ACT ~138us, PE ~145us busy — all near the ridge. Emitting wsum work
    earlier than one full group behind the score pipe measurably regresses
    the schedule, hence the graded group sizes instead.
  - Per-(chunk, group) partials land in r_all slots; one tiny reduce per
    slide folds them; host divides by the denominator.
"""

import sys

sys.path.insert(0, "/opt/trn_rl_repo")

import numpy as np
import ml_dtypes

import concourse.bacc as bacc
import concourse.tile as tile
from concourse import mybir
from concourse.bass_utils import run_bass_kernel_spmd

BF16 = ml_dtypes.bfloat16
B, N, D, H = 16, 8192, 1024, 256
NCORES = 8
SPC = B // NCORES          # slides per core
BLK = 512                  # rows per block (one score half / one DMA)
NBLK = N // BLK            # 16 blocks per slide
KCH = D // 128             # d-chunks (contraction blocks)
HCH = H // 128             # h-chunks
GROUPS = [4, 4, 4, 4]      # blocks per weighted-sum group (sum = NBLK)
GMAX = max(GROUPS)
SLOTS = len(GROUPS) + 1    # partial-sum slots (last group uses two)
NDVE = 5                   # weighted-sum chunks via DVE STT; rest premul+ACT

_NC_CACHE = {}


def _build_nc():
    bf = mybir.dt.bfloat16
    f32 = mybir.dt.float32
    AF = mybir.ActivationFunctionType
    ALU = mybir.AluOpType

    nc = bacc.Bacc("TRN2", num_devices=NCORES)
    # Host-swizzled 512-row blocks; each block DMA reads one contiguous
    # 1 MiB region into 128 x 8 KiB.
    #   xt[s, b, p, k*BLK + j] = X[s, b*BLK + j, k*128 + p]
    xt = nc.declare_dram_parameter("xt", [SPC, NBLK, 128, KCH * BLK], bf, isOutput=False)
    #   w1t[p, k*H + h] = W1[h, k*128 + p]
    w1t = nc.declare_dram_parameter("w1t", [128, KCH * H], bf, isOutput=False)
    #   w2rep[p, hc*128 + m] = W2[0, hc*128 + p]  (column-replicated blocks)
    w2rep = nc.declare_dram_parameter("w2rep", [128, HCH * 128], bf, isOutput=False)
    # out[s, p, k] = sum_n e_n X[n, k*128+p]  for k<8;  out[s, p, 8] = l
    outp = nc.declare_dram_parameter("out", [SPC, 128, KCH + 1], f32, isOutput=True)

    with tile.TileContext(nc) as tc:
        with tc.tile_pool(name="const", bufs=1) as constp, \
             tc.tile_pool(name="xt", bufs=4) as xtp, \
             tc.tile_pool(name="th", bufs=3) as thp, \
             tc.tile_pool(name="ebc", bufs=3) as ebcp, \
             tc.tile_pool(name="tmp3", bufs=2) as tmp3p, \
             tc.tile_pool(name="scr", bufs=1) as scrp, \
             tc.tile_pool(name="acc", bufs=1) as accp, \
             tc.tile_pool(name="osb", bufs=2) as osbp, \
             tc.tile_pool(name="htps", bufs=2, space="PSUM") as htpsp, \
             tc.tile_pool(name="warmps", bufs=1, space="PSUM") as warmpsp, \
             tc.tile_pool(name="sbc", bufs=3, space="PSUM") as sbcp:

            w1t_sb = constp.tile([128, KCH * H], bf)
            nc.gpsimd.dma_start(w1t_sb[:], w1t[:, :])
            w2rep_sb = constp.tile([128, HCH * 128], bf)
            nc.gpsimd.dma_start(w2rep_sb[:], w2rep[:, :])

            # p-state ramp: keep the PE busy ~3us so it reaches full clock
            # before the real matmuls (overlaps the first xt DMA).
            warm_sb = constp.tile([128, 256], bf)
            nc.gpsimd.memset(warm_sb[:], 0.0)
            warm_ps = warmpsp.tile([128, 512], f32)
            for _ in range(14):
                nc.tensor.matmul(
                    warm_ps[:, 0:256], warm_sb[:, 0:128], warm_sb[:, 0:256],
                    start=True, stop=True, skip_group_check=True,
                )

            # scratch for op outputs whose only consumed result is accum_out
            scr_dve = scrp.tile([128, GMAX * BLK], bf)
            scr_act = scrp.tile([128, GMAX * BLK], bf)

            for s in range(SPC):
                # per-(chunk, group) weighted-sum partials + per-block exp sums
                r_all = accp.tile([128, KCH * SLOTS], f32, tag=f"racc{s}")
                l_all = accp.tile([128, NBLK], f32, tag=f"lacc{s}")

                # score tail for one 512-row block: w2-matmul + exp
                def score_tail(u, i, th_sb, e_bc):
                    s_bc = sbcp.tile([128, BLK], f32)
                    for hc in range(HCH):
                        nc.tensor.matmul(
                            s_bc[:],
                            w2rep_sb[:, hc * 128:(hc + 1) * 128],
                            th_sb[:, hc * BLK:(hc + 1) * BLK],
                            start=(hc == 0), stop=(hc == HCH - 1),
                        )
                    nc.scalar.activation(
                        e_bc[:, i * BLK:(i + 1) * BLK], s_bc[:], AF.Exp,
                        accum_out=l_all[:, u:u + 1],
                    )

                # weighted-sum ops for one group of nb blocks. DVE work
                # (premultiplies first so ACT's inputs are ready early, then
                # the fused mul+reduce STTs) is emitted here; the ACT reduces
                # go on a backlog flushed one block later so tanh/exp keep
                # queue priority on the scalar engine.
                act_backlog = []

                def wsum(slot, xt_sb, e_bc, nb, b_off=0):
                    gw = nb * BLK
                    xv = xt_sb[:, b_off * KCH * BLK:(b_off + nb) * KCH * BLK].rearrange(
                        "p (b c) -> p b c", b=nb)
                    ev = e_bc[:, b_off * BLK:(b_off + nb) * BLK].rearrange(
                        "p (b j) -> p b j", b=nb)
                    tmp3 = tmp3p.tile([128, (KCH - NDVE) * GMAX * BLK], bf,
                                      name="tmp3")
                    for i in range(KCH - NDVE):
                        nc.vector.tensor_mul(
                            tmp3[:, i * gw:(i + 1) * gw].rearrange(
                                "p (b j) -> p b j", b=nb),
                            xv[:, :, (NDVE + i) * BLK:(NDVE + i + 1) * BLK],
                            ev,
                        )
                    act_backlog.append((slot, tmp3, nb))
                    for k in range(NDVE):
                        nc.vector.scalar_tensor_tensor(
                            out=scr_dve[:, 0:gw].rearrange(
                                "p (b j) -> p b j", b=nb),
                            in0=xv[:, :, k * BLK:(k + 1) * BLK],
                            scalar=1.0,
                            in1=ev,
                            op0=ALU.mult,
                            op1=ALU.mult,
                            accum_out=r_all[:, k * SLOTS + slot:k * SLOTS + slot + 1],
                        )

                def flush_act():
                    for slot, tmp3, nb in act_backlog:
                        gw = nb * BLK
                        for i in range(KCH - NDVE):
                            k = NDVE + i
                            nc.scalar.activation(
                                scr_act[:, 0:gw],
                                tmp3[:, i * gw:(i + 1) * gw],
                                AF.Copy,
                                accum_out=r_all[:, k * SLOTS + slot:k * SLOTS + slot + 1],
                            )
                    act_backlog.clear()

                pend_score = None   # block awaiting w2-matmul + exp
                pend_wsum = None    # group awaiting weighted-sum ops
                b0 = 0
                for gi, nb in enumerate(GROUPS):
                    xt_sb = xtp.tile([128, GMAX * KCH * BLK], bf, name="xt_sb")
                    for i in range(nb):
                        nc.sync.dma_start(
                            xt_sb[:, i * KCH * BLK:(i + 1) * KCH * BLK],
                            xt[s, b0 + i],
                        )
                    e_bc = ebcp.tile([128, GMAX * BLK], bf, name="e_bc")
                    for i in range(nb):
                        ht_ps = htpsp.tile([128, HCH * BLK], f32)
                        for hc in range(HCH):
                            for k in range(KCH):
                                nc.tensor.matmul(
                                    ht_ps[:, hc * BLK:(hc + 1) * BLK],
                                    w1t_sb[:, k * H + hc * 128: k * H + hc * 128 + 128],
                                    xt_sb[:, i * KCH * BLK + k * BLK: i * KCH * BLK + (k + 1) * BLK],
                                    start=(k == 0), stop=(k == KCH - 1),
                                )
                        th_sb = thp.tile([128, HCH * BLK], bf, name="th_sb")
                        nc.scalar.activation(th_sb[:], ht_ps[:], AF.Tanh)
                        if pend_score is not None:
                            score_tail(*pend_score)
                        pend_score = (b0 + i, i, th_sb, e_bc)
                        if pend_wsum is not None:
                            wsum(*pend_wsum)
                            pend_wsum = None
                        else:
                            flush_act()
                        if gi == len(GROUPS) - 1 and i == nb - 1:
                            # drain shrink: the final group's first 3 blocks
                            # can start as soon as their exps are done; only
                            # a single block's weighted sum remains at the end
                            wsum(len(GROUPS) - 1, xt_sb, e_bc, nb - 1, 0)
                    if gi < len(GROUPS) - 1:
                        pend_wsum = (gi, xt_sb, e_bc, nb)
                    b0 += nb
                score_tail(*pend_score)
                flush_act()
                wsum(len(GROUPS), xt_sb, e_bc, 1, nb - 1)
                flush_act()

                o_sb = osbp.tile([128, KCH + 1], f32)
                nc.vector.reduce_sum(
                    o_sb[:, 0:KCH],
                    r_all[:].rearrange("p (k t) -> p k t", k=KCH),
                    axis=mybir.AxisListType.X,
                )
                nc.vector.reduce_sum(
                    o_sb[:, KCH:KCH + 1],
                    l_all[:].rearrange("p (o t) -> p o t", o=1),
                    axis=mybir.AxisListType.X,
                )
                nc.scalar.dma_start(outp[s], o_sb[:])

    nc.compile()
    return nc


def _get_nc():
    if "nc" not in _NC_CACHE:
        _NC_CACHE["nc"] = _build_nc()
    return _NC_CACHE["nc"]


def _prep_inputs(tiles_embeddings, W1, W2):
    X_bf = tiles_embeddings.astype(BF16)
    # xt[b, blk, p, k, j] = X[b, blk*BLK + j, k*128 + p]
    xt_sw = np.ascontiguousarray(
        X_bf.reshape(B, NBLK, BLK, KCH, 128).transpose(0, 1, 4, 3, 2)
    ).reshape(B, NBLK, 128, KCH * BLK)
    # w1t[p, k, h] = W1[h, k*128 + p]
    w1t = np.ascontiguousarray(
        W1.astype(BF16).reshape(H, KCH, 128).transpose(2, 1, 0)
    ).reshape(128, KCH * H)
    # w2rep[p, hc*128 + m] = W2[0, hc*128 + p]
    w2c = W2.astype(BF16).reshape(HCH, 128)
    w2rep = np.ascontiguousarray(
        np.repeat(w2c[:, :, None], 128, axis=2).transpose(1, 0, 2)
    ).reshape(128, HCH * 128)
    return [
        {
            "xt": xt_sw[c * SPC:(c + 1) * SPC],
            "w1t": w1t,
            "w2rep": w2rep,
        }
        for c in range(NCORES)
    ]


def _run(tiles_embeddings, W1, W2, **spmd_kwargs):
    nc = _get_nc()
    in_maps = _prep_inputs(tiles_embeddings, W1, W2)
    res = run_bass_kernel_spmd(nc, in_maps, core_ids=list(range(NCORES)), **spmd_kwargs)
    raw = np.concatenate([r["out"] for r in res.results], axis=0)  # [B, 128, 9]
    acc = raw[:, :, 0:KCH].transpose(0, 2, 1).reshape(B, D)        # d = k*128 + p
    l = raw[:, 0, KCH]                                             # [B]
    out = acc / l[:, None]
    return out.astype(np.float32, copy=False), res


def kernel(tiles_embeddings, W1, W2):
    out, _ = _run(
        np.asarray(tiles_embeddings), np.asarray(W1), np.asarray(W2)
    )
    return out


# revision 21
# speedup vs baseline: 1.0153x; 1.0153x over previous
"""AttentionMIL pooling kernel for 8 Trainium2 NeuronCores.

Math (per slide b): h = tanh(X @ W1^T); s = h @ w2; a = softmax(s);
out = a^T @ X, with X [N=8192, D=1024], W1 [H=256, D], w2 [H].

Strategy (single-copy, ~33.6 MB/core HBM vs 67 MB for the two-layout
baseline; measured 221.8us baseline -> ~171us -> this version):
  - Data-parallel over the slide dim: 16 slides / 8 cores = 2 per core.
  - Host ships ONLY the transposed bf16 layout xt (d on partitions, rows on
    free), swizzled as 512-row blocks. The kernel assembles blocks into
    weighted-sum groups of GRADED size [2,4,4,4,2] blocks: small groups at
    the pipeline ramp and drain (the weighted sum trails the score pipe by
    one group, so group length = pipeline skew), big groups in the middle
    (amortize fixed per-op costs on DVE/ACT). Each block is one 1 MiB DMA
    so the PE starts on the first block.
  - Scores per 512-row block: ht = W1t-stationary @ xt on PE (16 matmuls of
    F=512 into a double-buffered 2-bank fp32 PSUM tile), one tanh (ACT).
    The w2 contraction is a PE matmul whose stationary is w2 REPLICATED
    across 128 columns: out [128, 512] has every partition equal to the
    score row, i.e. scores arrive already broadcast across partitions for
    the same cost (matmul cost depends only on F). exp on ACT fuses the
    PSUM->SBUF copy and emits accum_out = sum_j exp(s_j) (softmax
    denominator; no max-subtraction needed: |s| <= ||w2||_1 ~ 13 cannot
    overflow fp32). The score tail for block i is emitted after the ht
    matmuls of block i+1 so the PE never stalls waiting on ACT.
  - Weighted sum WITHOUT a second X layout: per d-chunk k and group,
    r[p, k] += sum_j xt[p, k-chunk, j] * e[j] over the group's blocks via
    strided 3D access patterns. Chunks 0-4 via fused scalar_tensor_tensor
    on DVE (mul + free-dim reduce + [P,1] accum in one 1x op). Chunks 5-7:
    2x tensor_tensor premultiplies on DVE, then three activation-accumulate
    reduces on ACT, deferred one block so tanh/exp keep priority in the
    scalar engine's in-order queue. Measured steady state: DVE ~145us,
    ACT ~138us, PE ~145us busy — all near the ridge. Emitting wsum work
    earlier than one full group behind the score pipe measurably regresses
    the schedule, hence the graded group sizes instead.
  - Per-(chunk, group) partials land in r_all slots; one tiny reduce per
    slide folds them; host divides by the denominator.
"""

import sys

sys.path.insert(0, "/opt/trn_rl_repo")

import numpy as np
import ml_dtypes

import concourse.bacc as bacc
import concourse.tile as tile
from concourse import mybir
from concourse.bass_utils import run_bass_kernel_spmd

BF16 = ml_dtypes.bfloat16
B, N, D, H = 16, 8192, 1024, 256
NCORES = 8
SPC = B // NCORES          # slides per core
BLK = 512                  # rows per block (one score half / one DMA)
NBLK = N // BLK            # 16 blocks per slide
KCH = D // 128             # d-chunks (contraction blocks)
HCH = H // 128             # h-chunks
GROUPS = [4, 4, 4, 4]      # blocks per weighted-sum group (sum = NBLK)
GMAX = max(GROUPS)
SLOTS = len(GROUPS) + 1    # partial-sum slots (last group uses two)
NDVE = 5                   # weighted-sum chunks via DVE STT; rest premul+ACT

_NC_CACHE = {}


def _build_nc():
    bf = mybir.dt.bfloat16
    f32 = mybir.dt.float32
    AF = mybir.ActivationFunctionType
    ALU = mybir.AluOpType

    nc = bacc.Bacc("TRN2", num_devices=NCORES)
    # Host-swizzled 512-row blocks; each block DMA reads one contiguous
    # 1 MiB region into 128 x 8 KiB.
    #   xt[s, b, p, k*BLK + j] = X[s, b*BLK + j, k*128 + p]
    xt = nc.declare_dram_parameter("xt", [SPC, NBLK, 128, KCH * BLK], bf, isOutput=False)
    #   w1t[p, k*H + h] = W1[h, k*128 + p]
    w1t = nc.declare_dram_parameter("w1t", [128, KCH * H], bf, isOutput=False)
    #   w2rep[p, hc*128 + m] = W2[0, hc*128 + p]  (column-replicated blocks)
    w2rep = nc.declare_dram_parameter("w2rep", [128, HCH * 128], bf, isOutput=False)
    # out[s, p, k] = sum_n e_n X[n, k*128+p]  for k<8;  out[s, p, 8] = l
    outp = nc.declare_dram_parameter("out", [SPC, 128, KCH + 1], f32, isOutput=True)

    with tile.TileContext(nc) as tc:
        with tc.tile_pool(name="const", bufs=1) as constp, \
             tc.tile_pool(name="xt", bufs=3) as xtp, \
             tc.tile_pool(name="th", bufs=3) as thp, \
             tc.tile_pool(name="ebc", bufs=3) as ebcp, \
             tc.tile_pool(name="tmp3", bufs=2) as tmp3p, \
             tc.tile_pool(name="scr", bufs=1) as scrp, \
             tc.tile_pool(name="acc", bufs=1) as accp, \
             tc.tile_pool(name="osb", bufs=2) as osbp, \
             tc.tile_pool(name="htps", bufs=2, space="PSUM") as htpsp, \
             tc.tile_pool(name="warmps", bufs=1, space="PSUM") as warmpsp, \
             tc.tile_pool(name="sbc", bufs=2, space="PSUM") as sbcp:

            w1t_sb = constp.tile([128, KCH * H], bf)
            nc.gpsimd.dma_start(w1t_sb[:], w1t[:, :])
            w2rep_sb = constp.tile([128, HCH * 128], bf)
            nc.gpsimd.dma_start(w2rep_sb[:], w2rep[:, :])

            # p-state ramp: keep the PE busy ~3us so it reaches full clock
            # before the real matmuls (overlaps the first xt DMA).
            warm_sb = constp.tile([128, 256], bf)
            nc.gpsimd.memset(warm_sb[:], 0.0)
            warm_ps = warmpsp.tile([128, 512], f32)
            for _ in range(14):
                nc.tensor.matmul(
                    warm_ps[:, 0:256], warm_sb[:, 0:128], warm_sb[:, 0:256],
                    start=True, stop=True, skip_group_check=True,
                )

            # scratch for op outputs whose only consumed result is accum_out
            scr_dve = scrp.tile([128, GMAX * BLK], bf)
            scr_act = scrp.tile([128, GMAX * BLK], bf)

            for s in range(SPC):
                # per-(chunk, group) weighted-sum partials + per-block exp sums
                r_all = accp.tile([128, KCH * SLOTS], f32, tag=f"racc{s}")
                l_all = accp.tile([128, NBLK], f32, tag=f"lacc{s}")

                # score tail for one 512-row block: w2-matmul + exp
                def score_tail(u, i, th_sb, e_bc):
                    s_bc = sbcp.tile([128, BLK], f32)
                    for hc in range(HCH):
                        nc.tensor.matmul(
                            s_bc[:],
                            w2rep_sb[:, hc * 128:(hc + 1) * 128],
                            th_sb[:, hc * BLK:(hc + 1) * BLK],
                            start=(hc == 0), stop=(hc == HCH - 1),
                        )
                    nc.scalar.activation(
                        e_bc[:, i * BLK:(i + 1) * BLK], s_bc[:], AF.Exp,
                        accum_out=l_all[:, u:u + 1],
                    )

                # weighted-sum ops for one group of nb blocks. DVE work
                # (premultiplies first so ACT's inputs are ready early, then
                # the fused mul+reduce STTs) is emitted here; the ACT reduces
                # go on a backlog flushed one block later so tanh/exp keep
                # queue priority on the scalar engine.
                act_backlog = []

                def wsum(slot, xt_sb, e_bc, nb, b_off=0):
                    gw = nb * BLK
                    xv = xt_sb[:, b_off * KCH * BLK:(b_off + nb) * KCH * BLK].rearrange(
                        "p (b c) -> p b c", b=nb)
                    ev = e_bc[:, b_off * BLK:(b_off + nb) * BLK].rearrange(
                        "p (b j) -> p b j", b=nb)
                    tmp3 = tmp3p.tile([128, (KCH - NDVE) * GMAX * BLK], bf,
                                      name="tmp3")
                    for i in range(KCH - NDVE):
                        nc.vector.tensor_mul(
                            tmp3[:, i * gw:(i + 1) * gw].rearrange(
                                "p (b j) -> p b j", b=nb),
                            xv[:, :, (NDVE + i) * BLK:(NDVE + i + 1) * BLK],
                            ev,
                        )
                    act_backlog.append((slot, tmp3, nb))
                    for k in range(NDVE):
                        nc.vector.scalar_tensor_tensor(
                            out=scr_dve[:, 0:gw].rearrange(
                                "p (b j) -> p b j", b=nb),
                            in0=xv[:, :, k * BLK:(k + 1) * BLK],
                            scalar=1.0,
                            in1=ev,
                            op0=ALU.mult,
                            op1=ALU.mult,
                            accum_out=r_all[:, k * SLOTS + slot:k * SLOTS + slot + 1],
                        )

                def flush_act():
                    for slot, tmp3, nb in act_backlog:
                        gw = nb * BLK
                        for i in range(KCH - NDVE):
                            k = NDVE + i
                            nc.scalar.activation(
                                scr_act[:, 0:gw],
                                tmp3[:, i * gw:(i + 1) * gw],
                                AF.Copy,
                                accum_out=r_all[:, k * SLOTS + slot:k * SLOTS + slot + 1],
                            )
                    act_backlog.clear()

                pend_score = None   # block awaiting w2-matmul + exp
                pend_wsum = None    # group awaiting weighted-sum ops
                b0 = 0
                for gi, nb in enumerate(GROUPS):
                    xt_sb = xtp.tile([128, GMAX * KCH * BLK], bf, name="xt_sb")
                    for i in range(nb):
                        nc.sync.dma_start(
                            xt_sb[:, i * KCH * BLK:(i + 1) * KCH * BLK],
                            xt[s, b0 + i],
                        )
                    e_bc = ebcp.tile([128, GMAX * BLK], bf, name="e_bc")
                    for i in range(nb):
                        ht_ps = htpsp.tile([128, HCH * BLK], f32)
                        for hc in range(HCH):
                            for k in range(KCH):
                                nc.tensor.matmul(
                                    ht_ps[:, hc * BLK:(hc + 1) * BLK],
                                    w1t_sb[:, k * H + hc * 128: k * H + hc * 128 + 128],
                                    xt_sb[:, i * KCH * BLK + k * BLK: i * KCH * BLK + (k + 1) * BLK],
                                    start=(k == 0), stop=(k == KCH - 1),
                                )
                        th_sb = thp.tile([128, HCH * BLK], bf, name="th_sb")
                        nc.scalar.activation(th_sb[:], ht_ps[:], AF.Tanh)
                        if pend_score is not None:
                            score_tail(*pend_score)
                        pend_score = (b0 + i, i, th_sb, e_bc)
                        if pend_wsum is not None:
                            wsum(*pend_wsum)
                            pend_wsum = None
                        else:
                            flush_act()
                        if gi == len(GROUPS) - 1 and i == nb - 1:
                            # drain shrink: the final group's first 3 blocks
                            # can start as soon as their exps are done; only
                            # a single block's weighted sum remains at the end
                            wsum(len(GROUPS) - 1, xt_sb, e_bc, nb - 1, 0)
                    if gi < len(GROUPS) - 1:
                        pend_wsum = (gi, xt_sb, e_bc, nb)
                    b0 += nb
                score_tail(*pend_score)
                flush_act()
                wsum(len(GROUPS), xt_sb, e_bc, 1, nb - 1)
                flush_act()

                o_sb = osbp.tile([128, KCH + 1], f32)
                nc.vector.reduce_sum(
                    o_sb[:, 0:KCH],
                    r_all[:].rearrange("p (k t) -> p k t", k=KCH),
                    axis=mybir.AxisListType.X,
                )
                nc.vector.reduce_sum(
                    o_sb[:, KCH:KCH + 1],
                    l_all[:].rearrange("p (o t) -> p o t", o=1),
                    axis=mybir.AxisListType.X,
                )
                nc.scalar.dma_start(outp[s], o_sb[:])

    nc.compile()
    return nc


def _get_nc():
    if "nc" not in _NC_CACHE:
        _NC_CACHE["nc"] = _build_nc()
    return _NC_CACHE["nc"]


def _prep_inputs(tiles_embeddings, W1, W2):
    X_bf = tiles_embeddings.astype(BF16)
    # xt[b, blk, p, k, j] = X[b, blk*BLK + j, k*128 + p]
    xt_sw = np.ascontiguousarray(
        X_bf.reshape(B, NBLK, BLK, KCH, 128).transpose(0, 1, 4, 3, 2)
    ).reshape(B, NBLK, 128, KCH * BLK)
    # w1t[p, k, h] = W1[h, k*128 + p]
    w1t = np.ascontiguousarray(
        W1.astype(BF16).reshape(H, KCH, 128).transpose(2, 1, 0)
    ).reshape(128, KCH * H)
    # w2rep[p, hc*128 + m] = W2[0, hc*128 + p]
    w2c = W2.astype(BF16).reshape(HCH, 128)
    w2rep = np.ascontiguousarray(
        np.repeat(w2c[:, :, None], 128, axis=2).transpose(1, 0, 2)
    ).reshape(128, HCH * 128)
    return [
        {
            "xt": xt_sw[c * SPC:(c + 1) * SPC],
            "w1t": w1t,
            "w2rep": w2rep,
        }
        for c in range(NCORES)
    ]


def _run(tiles_embeddings, W1, W2, **spmd_kwargs):
    nc = _get_nc()
    in_maps = _prep_inputs(tiles_embeddings, W1, W2)
    res = run_bass_kernel_spmd(nc, in_maps, core_ids=list(range(NCORES)), **spmd_kwargs)
    raw = np.concatenate([r["out"] for r in res.results], axis=0)  # [B, 128, 9]
    acc = raw[:, :, 0:KCH].transpose(0, 2, 1).reshape(B, D)        # d = k*128 + p
    l = raw[:, 0, KCH]                                             # [B]
    out = acc / l[:, None]
    return out.astype(np.float32, copy=False), res


def kernel(tiles_embeddings, W1, W2):
    out, _ = _run(
        np.asarray(tiles_embeddings), np.asarray(W1), np.asarray(W2)
    )
    return out


# revision 23
# speedup vs baseline: 1.0159x; 1.0006x over previous
"""AttentionMIL pooling kernel for 8 Trainium2 NeuronCores.

Math (per slide b): h = tanh(X @ W1^T); s = h @ w2; a = softmax(s);
out = a^T @ X, with X [N=8192, D=1024], W1 [H=256, D], w2 [H].

Strategy (single-copy, ~33.6 MB/core HBM vs 67 MB for the two-layout
baseline; measured 221.8us baseline -> ~171us -> this version):
  - Data-parallel over the slide dim: 16 slides / 8 cores = 2 per core.
  - Host ships ONLY the transposed bf16 layout xt (d on partitions, rows on
    free), swizzled as 512-row blocks. The kernel assembles blocks into
    weighted-sum groups of GRADED size [2,4,4,4,2] blocks: small groups at
    the pipeline ramp and drain (the weighted sum trails the score pipe by
    one group, so group length = pipeline skew), big groups in the middle
    (amortize fixed per-op costs on DVE/ACT). Each block is one 1 MiB DMA
    so the PE starts on the first block.
  - Scores per 512-row block: ht = W1t-stationary @ xt on PE (16 matmuls of
    F=512 into a double-buffered 2-bank fp32 PSUM tile), one tanh (ACT).
    The w2 contraction is a PE matmul whose stationary is w2 REPLICATED
    across 128 columns: out [128, 512] has every partition equal to the
    score row, i.e. scores arrive already broadcast across partitions for
    the same cost (matmul cost depends only on F). exp on ACT fuses the
    PSUM->SBUF copy and emits accum_out = sum_j exp(s_j) (softmax
    denominator; no max-subtraction needed: |s| <= ||w2||_1 ~ 13 cannot
    overflow fp32). The score tail for block i is emitted after the ht
    matmuls of block i+1 so the PE never stalls waiting on ACT.
  - Weighted sum WITHOUT a second X layout: per d-chunk k and group,
    r[p, k] += sum_j xt[p, k-chunk, j] * e[j] over the group's blocks via
    strided 3D access patterns. Chunks 0-4 via fused scalar_tensor_tensor
    on DVE (mul + free-dim reduce + [P,1] accum in one 1x op). Chunks 5-7:
    2x tensor_tensor premultiplies on DVE, then three activation-accumulate
    reduces on ACT, deferred one block so tanh/exp keep priority in the
    scalar engine's in-order queue. Measured steady state: DVE ~145us,
    ACT ~138us, PE ~145us busy — all near the ridge. Emitting wsum work
    earlier than one full group behind the score pipe measurably regresses
    the schedule, hence the graded group sizes instead.
  - Per-(chunk, group) partials land in r_all slots; one tiny reduce per
    slide folds them; host divides by the denominator.
"""

import sys

sys.path.insert(0, "/opt/trn_rl_repo")

import numpy as np
import ml_dtypes

import concourse.bacc as bacc
import concourse.tile as tile
from concourse import mybir
from concourse.bass_utils import run_bass_kernel_spmd

BF16 = ml_dtypes.bfloat16
B, N, D, H = 16, 8192, 1024, 256
NCORES = 8
SPC = B // NCORES          # slides per core
BLK = 512                  # rows per block (one score half / one DMA)
NBLK = N // BLK            # 16 blocks per slide
KCH = D // 128             # d-chunks (contraction blocks)
HCH = H // 128             # h-chunks
GROUPS = [4, 4, 4, 4]      # blocks per weighted-sum group (sum = NBLK)
GMAX = max(GROUPS)
SLOTS = len(GROUPS) + 1    # partial-sum slots (last group uses two)
NDVE = 5                   # weighted-sum chunks via DVE STT; rest premul+ACT

_NC_CACHE = {}


def _build_nc():
    bf = mybir.dt.bfloat16
    f32 = mybir.dt.float32
    AF = mybir.ActivationFunctionType
    ALU = mybir.AluOpType

    nc = bacc.Bacc("TRN2", num_devices=NCORES)
    # Host-swizzled 512-row blocks; each block DMA reads one contiguous
    # 1 MiB region into 128 x 8 KiB.
    #   xt[s, b, p, k*BLK + j] = X[s, b*BLK + j, k*128 + p]
    xt = nc.declare_dram_parameter("xt", [SPC, NBLK, 128, KCH * BLK], bf, isOutput=False)
    #   w1t[p, k*H + h] = W1[h, k*128 + p]
    w1t = nc.declare_dram_parameter("w1t", [128, KCH * H], bf, isOutput=False)
    #   w2rep[p, hc*128 + m] = W2[0, hc*128 + p]  (column-replicated blocks)
    w2rep = nc.declare_dram_parameter("w2rep", [128, HCH * 128], bf, isOutput=False)
    # out[s, p, k] = sum_n e_n X[n, k*128+p]
    outp = nc.declare_dram_parameter("out", [SPC, 128, KCH], f32, isOutput=True)
    # exp rows shipped to host (scores are partition-replicated; row 0
    # suffices) -- the softmax denominator is summed on the host from the
    # exact bf16 e values the device used, so the result is unchanged.
    e_out = nc.declare_dram_parameter("e_out", [SPC, NBLK, 1, BLK], bf, isOutput=True)

    with tile.TileContext(nc) as tc:
        with tc.tile_pool(name="const", bufs=1) as constp, \
             tc.tile_pool(name="xt", bufs=3) as xtp, \
             tc.tile_pool(name="th", bufs=3) as thp, \
             tc.tile_pool(name="ebc", bufs=4) as ebcp, \
             tc.tile_pool(name="tmp3", bufs=2) as tmp3p, \
             tc.tile_pool(name="scr", bufs=1) as scrp, \
             tc.tile_pool(name="acc", bufs=1) as accp, \
             tc.tile_pool(name="osb", bufs=2) as osbp, \
             tc.tile_pool(name="htps", bufs=2, space="PSUM") as htpsp, \
             tc.tile_pool(name="warmps", bufs=1, space="PSUM") as warmpsp, \
             tc.tile_pool(name="sbc", bufs=2, space="PSUM") as sbcp:

            w1t_sb = constp.tile([128, KCH * H], bf)
            nc.gpsimd.dma_start(w1t_sb[:], w1t[:, :])
            w2rep_sb = constp.tile([128, HCH * 128], bf)
            nc.gpsimd.dma_start(w2rep_sb[:], w2rep[:, :])

            # p-state ramp: keep the PE busy ~3us so it reaches full clock
            # before the real matmuls (overlaps the first xt DMA).
            warm_sb = constp.tile([128, 256], bf)
            nc.gpsimd.memset(warm_sb[:], 0.0)
            warm_ps = warmpsp.tile([128, 512], f32)
            for _ in range(14):
                nc.tensor.matmul(
                    warm_ps[:, 0:256], warm_sb[:, 0:128], warm_sb[:, 0:256],
                    start=True, stop=True, skip_group_check=True,
                )

            # scratch for op outputs whose only consumed result is accum_out
            scr_dve = scrp.tile([128, GMAX * BLK], bf)
            scr_act = scrp.tile([128, GMAX * BLK], bf)

            for s in range(SPC):
                # per-(chunk, group) weighted-sum partials + per-block exp sums
                r_all = accp.tile([128, KCH * SLOTS], f32, tag=f"racc{s}")

                # score tail for one 512-row block: w2-matmul + exp
                def score_tail(u, i, th_sb, e_bc):
                    s_bc = sbcp.tile([128, BLK], f32)
                    for hc in range(HCH):
                        nc.tensor.matmul(
                            s_bc[:],
                            w2rep_sb[:, hc * 128:(hc + 1) * 128],
                            th_sb[:, hc * BLK:(hc + 1) * BLK],
                            start=(hc == 0), stop=(hc == HCH - 1),
                        )
                    nc.scalar.activation(
                        e_bc[:, i * BLK:(i + 1) * BLK], s_bc[:], AF.Exp,
                    )
                    nc.sync.dma_start(e_out[s, u], e_bc[0:1, i * BLK:(i + 1) * BLK])

                # weighted-sum ops for one group of nb blocks. DVE work
                # (premultiplies first so ACT's inputs are ready early, then
                # the fused mul+reduce STTs) is emitted here; the ACT reduces
                # go on a backlog flushed one block later so tanh/exp keep
                # queue priority on the scalar engine.
                act_backlog = []

                def wsum(slot, xt_sb, e_bc, nb, b_off=0):
                    gw = nb * BLK
                    xv = xt_sb[:, b_off * KCH * BLK:(b_off + nb) * KCH * BLK].rearrange(
                        "p (b c) -> p b c", b=nb)
                    ev = e_bc[:, b_off * BLK:(b_off + nb) * BLK].rearrange(
                        "p (b j) -> p b j", b=nb)
                    tmp3 = tmp3p.tile([128, (KCH - NDVE) * GMAX * BLK], bf,
                                      name="tmp3")
                    for i in range(KCH - NDVE):
                        nc.vector.tensor_mul(
                            tmp3[:, i * gw:(i + 1) * gw].rearrange(
                                "p (b j) -> p b j", b=nb),
                            xv[:, :, (NDVE + i) * BLK:(NDVE + i + 1) * BLK],
                            ev,
                        )
                    act_backlog.append((slot, tmp3, nb))
                    for k in range(NDVE):
                        nc.vector.scalar_tensor_tensor(
                            out=scr_dve[:, 0:gw].rearrange(
                                "p (b j) -> p b j", b=nb),
                            in0=xv[:, :, k * BLK:(k + 1) * BLK],
                            scalar=1.0,
                            in1=ev,
                            op0=ALU.mult,
                            op1=ALU.mult,
                            accum_out=r_all[:, k * SLOTS + slot:k * SLOTS + slot + 1],
                        )

                def flush_act():
                    for slot, tmp3, nb in act_backlog:
                        gw = nb * BLK
                        for i in range(KCH - NDVE):
                            k = NDVE + i
                            nc.scalar.activation(
                                scr_act[:, 0:gw],
                                tmp3[:, i * gw:(i + 1) * gw],
                                AF.Copy,
                                accum_out=r_all[:, k * SLOTS + slot:k * SLOTS + slot + 1],
                            )
                    act_backlog.clear()

                pend_score = None   # block awaiting w2-matmul + exp
                pend_wsum = None    # group awaiting weighted-sum ops
                b0 = 0
                for gi, nb in enumerate(GROUPS):
                    xt_sb = xtp.tile([128, GMAX * KCH * BLK], bf, name="xt_sb")
                    for i in range(nb):
                        nc.sync.dma_start(
                            xt_sb[:, i * KCH * BLK:(i + 1) * KCH * BLK],
                            xt[s, b0 + i],
                        )
                    e_bc = ebcp.tile([128, GMAX * BLK], bf, name="e_bc")
                    for i in range(nb):
                        ht_ps = htpsp.tile([128, HCH * BLK], f32)
                        for hc in range(HCH):
                            for k in range(KCH):
                                nc.tensor.matmul(
                                    ht_ps[:, hc * BLK:(hc + 1) * BLK],
                                    w1t_sb[:, k * H + hc * 128: k * H + hc * 128 + 128],
                                    xt_sb[:, i * KCH * BLK + k * BLK: i * KCH * BLK + (k + 1) * BLK],
                                    start=(k == 0), stop=(k == KCH - 1),
                                )
                        th_sb = thp.tile([128, HCH * BLK], bf, name="th_sb")
                        nc.scalar.activation(th_sb[:], ht_ps[:], AF.Tanh)
                        if pend_score is not None:
                            score_tail(*pend_score)
                        pend_score = (b0 + i, i, th_sb, e_bc)
                        if pend_wsum is not None:
                            wsum(*pend_wsum)
                            pend_wsum = None
                        else:
                            flush_act()
                        if gi == len(GROUPS) - 1 and i == nb - 1:
                            # drain shrink: the final group's first half can
                            # start as soon as its first two exps are done
                            wsum(len(GROUPS) - 1, xt_sb, e_bc, nb // 2, 0)
                    if gi < len(GROUPS) - 1:
                        pend_wsum = (gi, xt_sb, e_bc, nb)
                    b0 += nb
                score_tail(*pend_score)
                flush_act()
                wsum(len(GROUPS), xt_sb, e_bc, nb - nb // 2, nb // 2)
                flush_act()

                o_sb = osbp.tile([128, KCH], f32)
                nc.vector.reduce_sum(
                    o_sb[:],
                    r_all[:].rearrange("p (k t) -> p k t", k=KCH),
                    axis=mybir.AxisListType.X,
                )
                nc.scalar.dma_start(outp[s], o_sb[:])

    nc.compile()
    return nc


def _get_nc():
    if "nc" not in _NC_CACHE:
        _NC_CACHE["nc"] = _build_nc()
    return _NC_CACHE["nc"]


def _prep_inputs(tiles_embeddings, W1, W2):
    X_bf = tiles_embeddings.astype(BF16)
    # xt[b, blk, p, k, j] = X[b, blk*BLK + j, k*128 + p]
    xt_sw = np.ascontiguousarray(
        X_bf.reshape(B, NBLK, BLK, KCH, 128).transpose(0, 1, 4, 3, 2)
    ).reshape(B, NBLK, 128, KCH * BLK)
    # w1t[p, k, h] = W1[h, k*128 + p]
    w1t = np.ascontiguousarray(
        W1.astype(BF16).reshape(H, KCH, 128).transpose(2, 1, 0)
    ).reshape(128, KCH * H)
    # w2rep[p, hc*128 + m] = W2[0, hc*128 + p]
    w2c = W2.astype(BF16).reshape(HCH, 128)
    w2rep = np.ascontiguousarray(
        np.repeat(w2c[:, :, None], 128, axis=2).transpose(1, 0, 2)
    ).reshape(128, HCH * 128)
    return [
        {
            "xt": xt_sw[c * SPC:(c + 1) * SPC],
            "w1t": w1t,
            "w2rep": w2rep,
        }
        for c in range(NCORES)
    ]


def _run(tiles_embeddings, W1, W2, **spmd_kwargs):
    nc = _get_nc()
    in_maps = _prep_inputs(tiles_embeddings, W1, W2)
    res = run_bass_kernel_spmd(nc, in_maps, core_ids=list(range(NCORES)), **spmd_kwargs)
    raw = np.concatenate([r["out"] for r in res.results], axis=0)  # [B, 128, 8]
    acc = raw.transpose(0, 2, 1).reshape(B, D)                     # d = k*128 + p
    e = np.concatenate([r["e_out"] for r in res.results], axis=0)  # [B, NBLK, 1, BLK]
    l = e.astype(np.float64).sum(axis=(1, 2, 3)).astype(np.float32)
    out = acc / l[:, None]
    return out.astype(np.float32, copy=False), res


def kernel(tiles_embeddings, W1, W2):
    out, _ = _run(
        np.asarray(tiles_embeddings), np.asarray(W1), np.asarray(W2)
    )
    return out
